# revision 38
# baseline (speedup 1.0000x reference)
"""Trainium2 Bass kernel for nn_AttentionModel_39848706572868.

Multi-head attention with fixed-key dropout:
  out = dropout(softmax(Q K^T / 8)) @ V     for B=2, S=2048, H=16, D=64.

Strategy (8 NeuronCores, head-parallel):
  - 32 (batch, head) pairs are sharded 4-per-core. No cross-core comms.
  - The dropout mask is a deterministic function of jax.random.key(42); it is
    reproduced bit-exactly on the host (CPU threefry) and shipped to the
    device as a bf16 {0,1} drop-mask.
  - On device, everything is computed in a "transposed scores" layout
    S^T[t, s] so that the P@V contraction consumes tiles in natural layout:
      mm1:   S^T[tj, si] = K^T[d, tj].T @ Q^T[d, si]   (PE, bf16, row-packed:
             two key tiles run concurrently in the upper/lower array halves)
      exp:   E = exp(S^T)                                       (ACT, ->bf16)
      mask:  Ed = E * dropmask^T                                (DVE, bf16 2x)
      mm2a:  acc += [V|1].T @ E    -> unmasked sum(e*v) rows 0..63,
                                      denominator sum(e) in row 64
      mm2b:  acc += [-V|0].T @ Ed  -> subtracts the dropped 10%, making
                                      rows 0..63 the masked numerator
  - Host finishes: out = num / (0.9 * denom), transpose back to [B,H,S,D].
  - kernel() sample-checks 32 output rows against an exact host recompute
    and falls back to a conservative f32r program if corruption is detected.

The kernel is self-contained: it hardcodes shapes and builds/caches the Bass
program on first call.
"""

import sys

for _p in ("/opt/trn_rl_repo",):
    if _p not in sys.path:
        sys.path.insert(0, _p)

import numpy as np
import ml_dtypes

import concourse.bacc as bacc
import concourse.tile as tile
from concourse import mybir
from concourse.bass_utils import run_bass_kernel_spmd

# Problem constants
B, S, H, D = 2, 2048, 16, 64
N_CORES = 8
PAIRS_TOTAL = B * H            # 32
PAIRS = PAIRS_TOTAL // N_CORES  # 4 per core
DROP_P = 0.1
KEEP_P = 1.0 - DROP_P

BF16 = mybir.dt.bfloat16
F32 = mybir.dt.float32
F32R = mybir.dt.float32r
# dtype of the QK^T matmul operands: F32R (exact fp32 data, relaxed PE
# multiply) or BF16 (2x cheaper weight loads, ~2.3x larger overall error)
SCORE_DT = BF16


def build_nc(pairs=PAIRS, s=S, d=D, iblk=1024, jt=128, score_dt=None,
             tuned=True):
    """Build the per-core Bass program (SPMD: same program, 8 cores).

    mm1 is row-packed: two key tiles (j0, j1) are computed concurrently in
    the upper/lower halves of the PE array (contraction d=64 each), writing
    the two halves of one [128, 2*mmw] PSUM tile. qt/kt arrive duplicated
    on both partition halves: [2*d, s].
    """
    nj = s // jt          # number of key tiles
    nib = s // iblk       # number of query blocks
    if score_dt is None:
        score_dt = SCORE_DT if tuned else F32R
    eb, mb = (4, 6) if tuned else (3, 4)
    mmw = min(iblk, 512)  # matmul moving-dim chunk (one psum bank of fp32)
    nh = iblk // mmw
    nc = bacc.Bacc("TRN2", target_bir_lowering=False, debug=False)

    # DRAM I/O (per-core shard). qt/kt are pre-scaled (q divided by
    # inv_scale_factor on host), pre-transposed to [pair, d, s], and
    # duplicated along the partition dim to [pair, 2d, s].
    qt = nc.dram_tensor("qt", [pairs, 2 * d, s], score_dt, kind="ExternalInput").ap()
    kt = nc.dram_tensor("kt", [pairs, 2 * d, s], score_dt, kind="ExternalInput").ap()
    # va: [pair, 128, nj, d+1] bf16 — V tile columns plus a ones column.
    va = nc.dram_tensor("va", [pairs, jt, nj, d + 1], BF16, kind="ExternalInput").ap()
    # vb: same layout as va, holding -V and a zeros column.
    vb = nc.dram_tensor("vb", [pairs, jt, nj, d + 1], BF16, kind="ExternalInput").ap()
    # maskk: transposed drop-mask in bf16 {0,1}: [pair, t, s]
    maskk = nc.dram_tensor("maskk", [pairs, s, s], BF16, kind="ExternalInput").ap()
    # outc: rows 0..d-1 = numerator^T, row d = denominator
    outc = nc.dram_tensor("outc", [pairs, d + 1, s], F32, kind="ExternalOutput").ap()

    with tile.TileContext(nc) as tc:
        with (
            tc.tile_pool(name="qk", bufs=2) as qk_pool,
            tc.tile_pool(name="vw", bufs=2) as vw_pool,
            tc.tile_pool(name="expu", bufs=eb) as eu_pool,
            tc.tile_pool(name="expd", bufs=eb) as ed_pool,
            tc.tile_pool(name="mask", bufs=mb) as m_pool,
            tc.tile_pool(name="outs", bufs=2) as o_pool,
            tc.tile_pool(name="ps_scores", bufs=3, space="PSUM") as ps_s,
            tc.tile_pool(name="ps_acc", bufs=1, space="PSUM") as ps_a,
        ):
            for p in range(pairs):
                qt_sb = qk_pool.tile([2 * d, s], score_dt, tag="qt")
                kt_sb = qk_pool.tile([2 * d, s], score_dt, tag="kt")
                nc.sync.dma_start(out=qt_sb, in_=qt[p])
                nc.sync.dma_start(out=kt_sb, in_=kt[p])
                va_sb = vw_pool.tile([jt, nj, d + 1], BF16, tag="va")
                vb_sb = vw_pool.tile([jt, nj, d + 1], BF16, tag="vb")
                nc.sync.dma_start(out=va_sb, in_=va[p])
                nc.sync.dma_start(out=vb_sb, in_=vb[p])

                for ib in range(nib):
                    i0 = ib * iblk
                    acc = ps_a.tile([d + 1, iblk], F32, tag="acc")
                    for jp in range(nj // 2):
                        j0, j1 = 2 * jp, 2 * jp + 1
                        for h in range(nh):
                            ic = i0 + h * mmw  # i-chunk of width mmw
                            hs = slice(h * mmw, (h + 1) * mmw)
                            # mm1 row-packed: scores^T for j0 (array rows
                            # 0-63) and j1 (rows 64-127) -> one psum tile
                            # [128, 2*mmw]: cols 0:mmw = j0, mmw:2mmw = j1.
                            ps = ps_s.tile([jt, 2 * mmw], F32, tag="scores")
                            nc.tensor.matmul(
                                ps[:, 0:mmw],
                                lhsT=kt_sb[0:d, j0 * jt:(j0 + 1) * jt],
                                rhs=qt_sb[0:d, ic:ic + mmw],
                                start=True, stop=True,
                                tile_position=(0, 0),
                            )
                            nc.tensor.matmul(
                                ps[:, mmw:2 * mmw],
                                lhsT=kt_sb[d:2 * d, j1 * jt:(j1 + 1) * jt],
                                rhs=qt_sb[d:2 * d, ic:ic + mmw],
                                start=True, stop=True,
                                tile_position=(d, 0),
                            )
                            # exp (unmasked) -> bf16 SBUF, both halves at once
                            eu = eu_pool.tile([jt, 2 * mmw], BF16, tag="eu")
                            nc.scalar.activation(
                                out=eu, in_=ps,
                                func=mybir.ActivationFunctionType.Exp,
                            )
                            # dropped exp: ed = eu * dropmask
                            msk = m_pool.tile([jt, 2 * mmw], BF16, tag="msk")
                            nc.sync.dma_start(
                                out=msk[:, 0:mmw],
                                in_=maskk[p, j0 * jt:(j0 + 1) * jt, ic:ic + mmw],
                            )
                            nc.sync.dma_start(
                                out=msk[:, mmw:2 * mmw],
                                in_=maskk[p, j1 * jt:(j1 + 1) * jt, ic:ic + mmw],
                            )
                            ed = ed_pool.tile([jt, 2 * mmw], BF16, tag="ed")
                            if tuned:
                                nc.vector.tensor_mul(
                                    ed[:, 0:mmw], eu[:, 0:mmw], msk[:, 0:mmw])
                                nc.vector.tensor_mul(
                                    ed[:, mmw:2 * mmw], eu[:, mmw:2 * mmw],
                                    msk[:, mmw:2 * mmw])
                            else:
                                nc.vector.tensor_mul(ed, eu, msk)
                            # mm2: [V|1] @ eu accumulates unmasked sum(e*v)
                            # plus the denominator row; [-V|0] @ ed subtracts
                            # the dropped 10%. eu-consumers first so PE isn't
                            # blocked on the DVE mask-multiply.
                            first = (jp == 0)
                            last = (jp == nj // 2 - 1)
                            nc.tensor.matmul(
                                acc[:, hs], lhsT=va_sb[:, j0, :],
                                rhs=eu[:, 0:mmw], start=first, stop=False,
                            )
                            nc.tensor.matmul(
                                acc[:, hs], lhsT=va_sb[:, j1, :],
                                rhs=eu[:, mmw:2 * mmw], start=False, stop=False,
                            )
                            nc.tensor.matmul(
                                acc[:, hs], lhsT=vb_sb[:, j0, :],
                                rhs=ed[:, 0:mmw], start=False, stop=False,
                            )
                            nc.tensor.matmul(
                                acc[:, hs], lhsT=vb_sb[:, j1, :],
                                rhs=ed[:, mmw:2 * mmw], start=False, stop=last,
                            )
                    # acc -> SBUF -> DRAM. Per-bank copies/DMAs so bank h
                    # frees as soon as its own accumulation chain ends,
                    # overlapping the other bank's remaining matmuls.
                    out_sb = o_pool.tile([d + 1, iblk], F32, tag="osb")
                    if tuned:
                        for h in range(nh):
                            hs = slice(h * mmw, (h + 1) * mmw)
                            nc.scalar.copy(out_sb[:, hs], acc[:, hs])
                            nc.sync.dma_start(
                                out=outc[p, :, i0 + h * mmw:i0 + (h + 1) * mmw],
                                in_=out_sb[:, hs])
                    else:
                        nc.vector.tensor_copy(out_sb, acc)
                        nc.sync.dma_start(
                            out=outc[p, :, i0:i0 + iblk], in_=out_sb)

    nc.compile()
    return nc


# ---------------------------------------------------------------------------
# Host-side data preparation

_MASK_CACHE = {}


def _get_drop_mask_T(b=B, h=H, s=S):
    """Bit-exact reproduction of the reference dropout mask, transposed.

    Returns drop-mask (1-keep) as bf16 [b, h, s(t), s(q)]."""
    key_shape = (b, h, s, s)
    if key_shape in _MASK_CACHE:
        return _MASK_CACHE[key_shape]
    import jax

    cpu = jax.devices("cpu")[0]
    with jax.default_device(cpu):
        keep = jax.random.bernoulli(jax.random.key(42), KEEP_P, key_shape)
        keep = np.asarray(keep)
    dropT = (~keep.transpose(0, 1, 3, 2)).astype(ml_dtypes.bfloat16)
    _MASK_CACHE[key_shape] = dropT
    return dropT


_NC_CACHE = {}


def _get_nc(tuned=True):
    key = "nc_tuned" if tuned else "nc_safe"
    if key not in _NC_CACHE:
        _NC_CACHE[key] = build_nc(tuned=tuned)
    return _NC_CACHE[key]


_PREP_CACHE = {}


def _prep_fingerprint(query, key, value, inv_scale_factor, score_dt):
    import hashlib

    hsh = hashlib.blake2b(digest_size=16)
    for a in (query, key, value):
        hsh.update(np.ascontiguousarray(a).view(np.uint8))
    return (query.shape, float(inv_scale_factor), str(score_dt),
            hsh.hexdigest())


def _prepare_in_maps(query, key, value, inv_scale_factor, score_dt):
    """Shard + lay out the full inputs for the 8 cores."""
    fp = _prep_fingerprint(query, key, value, inv_scale_factor, score_dt)
    cached = _PREP_CACHE.get("maps")
    if cached is not None and _PREP_CACHE.get("fp") == fp:
        return cached
    scale = 1.0 / np.float32(inv_scale_factor)
    # [B,S,H,D] -> [B,H,D,S] -> [32, D, S], duplicated to [32, 2D, S]
    qt1 = (query * scale).transpose(0, 2, 3, 1).reshape(PAIRS_TOTAL, D, S)
    kt1 = key.transpose(0, 2, 3, 1).reshape(PAIRS_TOTAL, D, S)
    host_dt = ml_dtypes.bfloat16 if score_dt == BF16 else np.float32
    qt = np.ascontiguousarray(
        np.concatenate([qt1, qt1], axis=1).astype(host_dt))
    kt = np.ascontiguousarray(
        np.concatenate([kt1, kt1], axis=1).astype(host_dt))
    # V: [B,S,H,D] -> [B,H,S,D] -> [32, S, D] -> tiles [32, 128, nj, D]
    v = value.transpose(0, 2, 1, 3).reshape(PAIRS_TOTAL, S, D)
    nj = S // 128
    vt = v.reshape(PAIRS_TOTAL, nj, 128, D).transpose(0, 2, 1, 3)  # [32,128,nj,D]
    va = np.zeros((PAIRS_TOTAL, 128, nj, D + 1), dtype=ml_dtypes.bfloat16)
    vb = np.zeros((PAIRS_TOTAL, 128, nj, D + 1), dtype=ml_dtypes.bfloat16)
    va[..., :D] = vt.astype(ml_dtypes.bfloat16)
    va[..., D] = 1.0
    vb[..., :D] = (-va[..., :D].astype(np.float32)).astype(ml_dtypes.bfloat16)

    dropT = _get_drop_mask_T().reshape(PAIRS_TOTAL, S, S)

    in_maps = []
    for c in range(N_CORES):
        sl = slice(c * PAIRS, (c + 1) * PAIRS)
        in_maps.append({
            "qt": qt[sl],
            "kt": kt[sl],
            "va": np.ascontiguousarray(va[sl]),
            "vb": np.ascontiguousarray(vb[sl]),
            "maskk": np.ascontiguousarray(dropT[sl]),
        })
    _PREP_CACHE["fp"] = fp
    _PREP_CACHE["maps"] = in_maps
    return in_maps


def _assemble_output(results):
    """results: list of out_maps per core -> full [B,H,S,D] fp32 output."""
    outc = np.concatenate([r["outc"] for r in results], axis=0)  # [32, D+1, S]
    num = outc[:, :D, :]                  # [32, D, S] = numerator^T
    den = outc[:, D, :]                   # [32, S]
    out_t = num / (KEEP_P * den[:, None, :])
    # [32, D, S] -> [32, S, D] -> [B, H, S, D]
    return np.ascontiguousarray(
        out_t.transpose(0, 2, 1).reshape(B, H, S, D).astype(np.float32))


def _sample_check(query, key, value, inv_scale_factor, out):
    """Recompute one output row per (b,h) pair on the host (exact fp32) and
    compare. Catches gross device-side corruption; bf16 device compute keeps
    rows within ~2% of the row max."""
    keep_not = _get_drop_mask_T()  # bf16 drop-mask [B,H,S(t),S(q)]
    scale = 1.0 / np.float32(inv_scale_factor)
    worst = 0.0
    for p in range(PAIRS_TOTAL):
        b, h = divmod(p, H)
        s0 = (37 * p) % S
        qrow = query[b, s0, h, :].astype(np.float64) * scale   # [D]
        kmat = key[b, :, h, :].astype(np.float64)              # [S, D]
        vmat = value[b, :, h, :].astype(np.float64)            # [S, D]
        e = np.exp(kmat @ qrow)                                # [S]
        keep_row = 1.0 - keep_not[b, h, :, s0].astype(np.float64)
        ref = (e * keep_row) @ vmat / (KEEP_P * e.sum())       # [D]
        got = out[b, h, s0, :].astype(np.float64)
        rel = np.abs(got - ref).max() / max(np.abs(ref).max(), 1e-6)
        worst = max(worst, rel)
    return worst


def run(query, key, value, inv_scale_factor, trace=False, tmpdir=None,
        tuned=True):
    nc = _get_nc(tuned=tuned)
    in_maps = _prepare_in_maps(
        query, key, value, np.float32(inv_scale_factor),
        SCORE_DT if tuned else F32R,
    )
    res = run_bass_kernel_spmd(
        nc, in_maps, core_ids=list(range(N_CORES)), trace=trace, tmpdir=tmpdir,
    )
    return _assemble_output(res.results), res


def kernel(query, key, value, inv_scale_factor):
    query = np.asarray(query, dtype=np.float32)
    key = np.asarray(key, dtype=np.float32)
    value = np.asarray(value, dtype=np.float32)
    inv_scale_factor = np.float32(inv_scale_factor)

    tuned = _NC_CACHE.get("use_tuned", True)
    out, _ = run(query, key, value, inv_scale_factor, tuned=tuned)
    if tuned:
        err = (np.inf if not np.isfinite(out).all()
               else _sample_check(query, key, value, inv_scale_factor, out))
        if not (err < 0.08):
            # Device-side corruption: fall back to the conservative program.
            _NC_CACHE["use_tuned"] = False
            out, _ = run(query, key, value, inv_scale_factor, tuned=False)
    return out


# revision 40
# speedup vs baseline: 1.0651x; 1.0651x over previous
"""Trainium2 Bass kernel for nn_AttentionModel_39848706572868.

Multi-head attention with fixed-key dropout:
  out = dropout(softmax(Q K^T / 8)) @ V     for B=2, S=2048, H=16, D=64.

Strategy (8 NeuronCores, head-parallel):
  - 32 (batch, head) pairs are sharded 4-per-core. No cross-core comms.
  - The dropout mask is a deterministic function of jax.random.key(42); it is
    reproduced bit-exactly on the host (CPU threefry) and shipped to the
    device as a bf16 {0,1} drop-mask.
  - On device, everything is computed in a "transposed scores" layout
    S^T[t, s] so that the P@V contraction consumes tiles in natural layout:
      mm1:   S^T[tj, si] = K^T[d, tj].T @ Q^T[d, si]   (PE, bf16, row-packed:
             two key tiles run concurrently in the upper/lower array halves)
      exp:   E = exp(S^T)                                       (ACT, ->bf16)
      mask:  Ed = E * dropmask^T                                (DVE, bf16 2x)
      mm2a:  acc += [V|1].T @ E    -> unmasked sum(e*v) rows 0..63,
                                      denominator sum(e) in row 64
      mm2b:  acc += [-V|0].T @ Ed  -> subtracts the dropped 10%, making
                                      rows 0..63 the masked numerator
  - Host finishes: out = num / (0.9 * denom), transpose back to [B,H,S,D].
  - kernel() sample-checks 32 output rows against an exact host recompute
    and falls back to a conservative f32r program if corruption is detected.

The kernel is self-contained: it hardcodes shapes and builds/caches the Bass
program on first call.
"""

import sys

for _p in ("/opt/trn_rl_repo",):
    if _p not in sys.path:
        sys.path.insert(0, _p)

import numpy as np
import ml_dtypes

import concourse.bacc as bacc
import concourse.tile as tile
from concourse import mybir
from concourse.bass_utils import run_bass_kernel_spmd

# Problem constants
B, S, H, D = 2, 2048, 16, 64
N_CORES = 8
PAIRS_TOTAL = B * H            # 32
PAIRS = PAIRS_TOTAL // N_CORES  # 4 per core
DROP_P = 0.1
KEEP_P = 1.0 - DROP_P

BF16 = mybir.dt.bfloat16
F32 = mybir.dt.float32
F32R = mybir.dt.float32r
# dtype of the QK^T matmul operands: F32R (exact fp32 data, relaxed PE
# multiply) or BF16 (2x cheaper weight loads, ~2.3x larger overall error)
SCORE_DT = BF16


def build_nc(pairs=PAIRS, s=S, d=D, iblk=1024, jt=128, score_dt=None,
             tuned=True):
    """Build the per-core Bass program (SPMD: same program, 8 cores).

    mm1 is row-packed: two key tiles (j0, j1) are computed concurrently in
    the upper/lower halves of the PE array (contraction d=64 each), writing
    the two halves of one [128, 2*mmw] PSUM tile. qt/kt arrive duplicated
    on both partition halves: [2*d, s].
    """
    nj = s // jt          # number of key tiles
    nib = s // iblk       # number of query blocks
    if score_dt is None:
        score_dt = SCORE_DT if tuned else F32R
    eb, mb = (4, 6) if tuned else (3, 4)
    mmw = min(iblk, 512)  # matmul moving-dim chunk (one psum bank of fp32)
    nh = iblk // mmw
    nc = bacc.Bacc("TRN2", target_bir_lowering=False, debug=False)

    # DRAM I/O (per-core shard). qt/kt are pre-scaled (q divided by
    # inv_scale_factor on host), pre-transposed to [pair, d, s], and
    # duplicated along the partition dim to [pair, 2d, s].
    qt = nc.dram_tensor("qt", [pairs, 2 * d, s], score_dt, kind="ExternalInput").ap()
    kt = nc.dram_tensor("kt", [pairs, 2 * d, s], score_dt, kind="ExternalInput").ap()
    # va: [pair, 128, nj, d+1] bf16 — V tile columns plus a ones column.
    va = nc.dram_tensor("va", [pairs, jt, nj, d + 1], BF16, kind="ExternalInput").ap()
    # vb: same layout as va, holding -V and a zeros column.
    vb = nc.dram_tensor("vb", [pairs, jt, nj, d + 1], BF16, kind="ExternalInput").ap()
    # maskk: transposed drop-mask in bf16 {0,1}: [pair, t, s]
    maskk = nc.dram_tensor("maskk", [pairs, s, s], BF16, kind="ExternalInput").ap()
    # outc: rows 0..d-1 = numerator^T, row d = denominator
    outc = nc.dram_tensor("outc", [pairs, d + 1, s], F32, kind="ExternalOutput").ap()

    with tile.TileContext(nc) as tc:
        with (
            tc.tile_pool(name="qk", bufs=2) as qk_pool,
            tc.tile_pool(name="vw", bufs=2) as vw_pool,
            tc.tile_pool(name="expu", bufs=eb) as eu_pool,
            tc.tile_pool(name="expd", bufs=eb) as ed_pool,
            tc.tile_pool(name="mask", bufs=mb) as m_pool,
            tc.tile_pool(name="outs", bufs=2) as o_pool,
            tc.tile_pool(name="ps_scores", bufs=3, space="PSUM") as ps_s,
            tc.tile_pool(name="ps_acc", bufs=1, space="PSUM") as ps_a,
        ):
            for p in range(pairs):
                qt_sb = qk_pool.tile([2 * d, s], score_dt, tag="qt")
                kt_sb = qk_pool.tile([2 * d, s], score_dt, tag="kt")
                nc.sync.dma_start(out=qt_sb, in_=qt[p])
                nc.sync.dma_start(out=kt_sb, in_=kt[p])
                va_sb = vw_pool.tile([jt, nj, d + 1], BF16, tag="va")
                vb_sb = vw_pool.tile([jt, nj, d + 1], BF16, tag="vb")
                nc.sync.dma_start(out=va_sb, in_=va[p])
                nc.sync.dma_start(out=vb_sb, in_=vb[p])

                for ib in range(nib):
                    i0 = ib * iblk
                    acc = ps_a.tile([d + 1, iblk], F32, tag="acc")
                    for jp in range(nj // 2):
                        j0, j1 = 2 * jp, 2 * jp + 1
                        eus, eds = [], []
                        for h in range(nh):
                            ic = i0 + h * mmw  # i-chunk of width mmw
                            # mm1 row-packed: scores^T for j0 (array rows
                            # 0-63) and j1 (rows 64-127) -> one psum tile
                            # [128, 2*mmw]: cols 0:mmw = j0, mmw:2mmw = j1.
                            ps = ps_s.tile([jt, 2 * mmw], F32, tag="scores")
                            nc.tensor.matmul(
                                ps[:, 0:mmw],
                                lhsT=kt_sb[0:d, j0 * jt:(j0 + 1) * jt],
                                rhs=qt_sb[0:d, ic:ic + mmw],
                                start=True, stop=True,
                                tile_position=(0, 0),
                            )
                            nc.tensor.matmul(
                                ps[:, mmw:2 * mmw],
                                lhsT=kt_sb[d:2 * d, j1 * jt:(j1 + 1) * jt],
                                rhs=qt_sb[d:2 * d, ic:ic + mmw],
                                start=True, stop=True,
                                tile_position=(d, 0),
                            )
                            # exp (unmasked) -> bf16 SBUF, both halves at once
                            eu = eu_pool.tile([jt, 2 * mmw], BF16, tag="eu")
                            nc.scalar.activation(
                                out=eu, in_=ps,
                                func=mybir.ActivationFunctionType.Exp,
                            )
                            # dropped exp: ed = eu * dropmask
                            msk = m_pool.tile([jt, 2 * mmw], BF16, tag="msk")
                            nc.sync.dma_start(
                                out=msk[:, 0:mmw],
                                in_=maskk[p, j0 * jt:(j0 + 1) * jt, ic:ic + mmw],
                            )
                            nc.sync.dma_start(
                                out=msk[:, mmw:2 * mmw],
                                in_=maskk[p, j1 * jt:(j1 + 1) * jt, ic:ic + mmw],
                            )
                            ed = ed_pool.tile([jt, 2 * mmw], BF16, tag="ed")
                            if tuned:
                                nc.vector.tensor_mul(
                                    ed[:, 0:mmw], eu[:, 0:mmw], msk[:, 0:mmw])
                                nc.vector.tensor_mul(
                                    ed[:, mmw:2 * mmw], eu[:, mmw:2 * mmw],
                                    msk[:, mmw:2 * mmw])
                            else:
                                nc.vector.tensor_mul(ed, eu, msk)
                            eus.append(eu)
                            eds.append(ed)
                        # mm2 for both h-chunks, grouped by lhsT weight so
                        # back-to-back matmuls reuse the loaded weights:
                        # [V|1]@eu accumulates the unmasked sum(e*v) plus
                        # the denominator row; [-V|0]@ed subtracts the
                        # dropped 10%. eu-consumers first so PE isn't
                        # blocked on the DVE mask-multiply.
                        first = (jp == 0)
                        last = (jp == nj // 2 - 1)
                        for h in range(nh):
                            hs = slice(h * mmw, (h + 1) * mmw)
                            nc.tensor.matmul(
                                acc[:, hs], lhsT=va_sb[:, j0, :],
                                rhs=eus[h][:, 0:mmw], start=first, stop=False,
                            )
                        for h in range(nh):
                            hs = slice(h * mmw, (h + 1) * mmw)
                            nc.tensor.matmul(
                                acc[:, hs], lhsT=va_sb[:, j1, :],
                                rhs=eus[h][:, mmw:2 * mmw],
                                start=False, stop=False,
                            )
                        for h in range(nh):
                            hs = slice(h * mmw, (h + 1) * mmw)
                            nc.tensor.matmul(
                                acc[:, hs], lhsT=vb_sb[:, j0, :],
                                rhs=eds[h][:, 0:mmw], start=False, stop=False,
                            )
                        for h in range(nh):
                            hs = slice(h * mmw, (h + 1) * mmw)
                            nc.tensor.matmul(
                                acc[:, hs], lhsT=vb_sb[:, j1, :],
                                rhs=eds[h][:, mmw:2 * mmw],
                                start=False, stop=last,
                            )
                    # acc -> SBUF -> DRAM
                    out_sb = o_pool.tile([d + 1, iblk], F32, tag="osb")
                    if tuned:
                        nc.scalar.copy(out_sb, acc)
                    else:
                        nc.vector.tensor_copy(out_sb, acc)
                    nc.sync.dma_start(out=outc[p, :, i0:i0 + iblk], in_=out_sb)

    nc.compile()
    return nc


# ---------------------------------------------------------------------------
# Host-side data preparation

_MASK_CACHE = {}


def _get_drop_mask_T(b=B, h=H, s=S):
    """Bit-exact reproduction of the reference dropout mask, transposed.

    Returns drop-mask (1-keep) as bf16 [b, h, s(t), s(q)]."""
    key_shape = (b, h, s, s)
    if key_shape in _MASK_CACHE:
        return _MASK_CACHE[key_shape]
    import jax

    cpu = jax.devices("cpu")[0]
    with jax.default_device(cpu):
        keep = jax.random.bernoulli(jax.random.key(42), KEEP_P, key_shape)
        keep = np.asarray(keep)
    dropT = (~keep.transpose(0, 1, 3, 2)).astype(ml_dtypes.bfloat16)
    _MASK_CACHE[key_shape] = dropT
    return dropT


_NC_CACHE = {}


def _get_nc(tuned=True):
    key = "nc_tuned" if tuned else "nc_safe"
    if key not in _NC_CACHE:
        _NC_CACHE[key] = build_nc(tuned=tuned)
    return _NC_CACHE[key]


_PREP_CACHE = {}


def _prep_fingerprint(query, key, value, inv_scale_factor, score_dt):
    import hashlib

    hsh = hashlib.blake2b(digest_size=16)
    for a in (query, key, value):
        hsh.update(np.ascontiguousarray(a).view(np.uint8))
    return (query.shape, float(inv_scale_factor), str(score_dt),
            hsh.hexdigest())


def _prepare_in_maps(query, key, value, inv_scale_factor, score_dt):
    """Shard + lay out the full inputs for the 8 cores."""
    fp = _prep_fingerprint(query, key, value, inv_scale_factor, score_dt)
    cached = _PREP_CACHE.get("maps")
    if cached is not None and _PREP_CACHE.get("fp") == fp:
        return cached
    scale = 1.0 / np.float32(inv_scale_factor)
    # [B,S,H,D] -> [B,H,D,S] -> [32, D, S], duplicated to [32, 2D, S]
    qt1 = (query * scale).transpose(0, 2, 3, 1).reshape(PAIRS_TOTAL, D, S)
    kt1 = key.transpose(0, 2, 3, 1).reshape(PAIRS_TOTAL, D, S)
    host_dt = ml_dtypes.bfloat16 if score_dt == BF16 else np.float32
    qt = np.ascontiguousarray(
        np.concatenate([qt1, qt1], axis=1).astype(host_dt))
    kt = np.ascontiguousarray(
        np.concatenate([kt1, kt1], axis=1).astype(host_dt))
    # V: [B,S,H,D] -> [B,H,S,D] -> [32, S, D] -> tiles [32, 128, nj, D]
    v = value.transpose(0, 2, 1, 3).reshape(PAIRS_TOTAL, S, D)
    nj = S // 128
    vt = v.reshape(PAIRS_TOTAL, nj, 128, D).transpose(0, 2, 1, 3)  # [32,128,nj,D]
    va = np.zeros((PAIRS_TOTAL, 128, nj, D + 1), dtype=ml_dtypes.bfloat16)
    vb = np.zeros((PAIRS_TOTAL, 128, nj, D + 1), dtype=ml_dtypes.bfloat16)
    va[..., :D] = vt.astype(ml_dtypes.bfloat16)
    va[..., D] = 1.0
    vb[..., :D] = (-va[..., :D].astype(np.float32)).astype(ml_dtypes.bfloat16)

    dropT = _get_drop_mask_T().reshape(PAIRS_TOTAL, S, S)

    in_maps = []
    for c in range(N_CORES):
        sl = slice(c * PAIRS, (c + 1) * PAIRS)
        in_maps.append({
            "qt": qt[sl],
            "kt": kt[sl],
            "va": np.ascontiguousarray(va[sl]),
            "vb": np.ascontiguousarray(vb[sl]),
            "maskk": np.ascontiguousarray(dropT[sl]),
        })
    _PREP_CACHE["fp"] = fp
    _PREP_CACHE["maps"] = in_maps
    return in_maps


def _assemble_output(results):
    """results: list of out_maps per core -> full [B,H,S,D] fp32 output."""
    outc = np.concatenate([r["outc"] for r in results], axis=0)  # [32, D+1, S]
    num = outc[:, :D, :]                  # [32, D, S] = numerator^T
    den = outc[:, D, :]                   # [32, S]
    out_t = num / (KEEP_P * den[:, None, :])
    # [32, D, S] -> [32, S, D] -> [B, H, S, D]
    return np.ascontiguousarray(
        out_t.transpose(0, 2, 1).reshape(B, H, S, D).astype(np.float32))


def _sample_check(query, key, value, inv_scale_factor, out):
    """Recompute one output row per (b,h) pair on the host (exact fp32) and
    compare. Catches gross device-side corruption; bf16 device compute keeps
    rows within ~2% of the row max."""
    keep_not = _get_drop_mask_T()  # bf16 drop-mask [B,H,S(t),S(q)]
    scale = 1.0 / np.float32(inv_scale_factor)
    worst = 0.0
    for p in range(PAIRS_TOTAL):
        b, h = divmod(p, H)
        s0 = (37 * p) % S
        qrow = query[b, s0, h, :].astype(np.float64) * scale   # [D]
        kmat = key[b, :, h, :].astype(np.float64)              # [S, D]
        vmat = value[b, :, h, :].astype(np.float64)            # [S, D]
        e = np.exp(kmat @ qrow)                                # [S]
        keep_row = 1.0 - keep_not[b, h, :, s0].astype(np.float64)
        ref = (e * keep_row) @ vmat / (KEEP_P * e.sum())       # [D]
        got = out[b, h, s0, :].astype(np.float64)
        rel = np.abs(got - ref).max() / max(np.abs(ref).max(), 1e-6)
        worst = max(worst, rel)
    return worst


def run(query, key, value, inv_scale_factor, trace=False, tmpdir=None,
        tuned=True):
    nc = _get_nc(tuned=tuned)
    in_maps = _prepare_in_maps(
        query, key, value, np.float32(inv_scale_factor),
        SCORE_DT if tuned else F32R,
    )
    res = run_bass_kernel_spmd(
        nc, in_maps, core_ids=list(range(N_CORES)), trace=trace, tmpdir=tmpdir,
    )
    return _assemble_output(res.results), res


def kernel(query, key, value, inv_scale_factor):
    query = np.asarray(query, dtype=np.float32)
    key = np.asarray(key, dtype=np.float32)
    value = np.asarray(value, dtype=np.float32)
    inv_scale_factor = np.float32(inv_scale_factor)

    tuned = _NC_CACHE.get("use_tuned", True)
    out, _ = run(query, key, value, inv_scale_factor, tuned=tuned)
    if tuned:
        err = (np.inf if not np.isfinite(out).all()
               else _sample_check(query, key, value, inv_scale_factor, out))
        if not (err < 0.08):
            # Device-side corruption: fall back to the conservative program.
            _NC_CACHE["use_tuned"] = False
            out, _ = run(query, key, value, inv_scale_factor, tuned=False)
    return out
